# revision 33
# baseline (speedup 1.0000x reference)
"""Causal self-attention (QKV proj + RoPE + causal softmax attention +
output proj + LayerNorm) on 8 Trainium2 NeuronCores.

Sharding: core c handles batch b = c//4 and head group g = c%4 (4 of the
16 heads). Each core projects QKV for its heads from the full sequence,
runs flash-style attention in transposed ([dim, token]) layout, computes
its partial c_proj contribution, AllToAlls y head-pair slices in
128-token chunks (overlapped with later attention), LayerNorms each
received 128-token chunk, and returns [512, 1024] rows. Host assembles
the [2, 2048, 1024] output.

Precision: bf16 storage/matmuls throughout (f32 PSUM accumulate);
softmax/normalization statistics in f32.
"""
import sys

sys.path.insert(0, "/opt/trn_rl_repo")

import numpy as np

import concourse.bass as bass
import concourse.tile as tile
from concourse.tile_rust import add_dep_helper
from concourse import bacc, mybir
from concourse.bass_utils import run_bass_kernel_spmd

B, T, C, H = 2, 2048, 1024, 16
D = C // H              # 64
HPC = 4                 # heads per core
DC = HPC * D            # 256 local q/k/v dims per core
N_CORES = 8
EPS = 1e-5

F32 = mybir.dt.float32
BF16 = mybir.dt.bfloat16

TT = 512                # token tile (moving dim) for qkv/attention
NTT = T // TT           # 4
NTB = T // 128          # 16 token blocks


def build(use_lnwb):
    nc = bacc.Bacc("TRN2", target_bir_lowering=False, debug=False,
                   num_devices=N_CORES)

    # DMA-friendly layouts: per-SBUF-partition lines are contiguous in DRAM
    xT_d = nc.dram_tensor("xT", [128, NTT, 8, TT], BF16,
                          kind="ExternalInput").ap()
    # column-block major: [part, colblock(6), kc(8), col(128)] so the first
    # qkv column block (all kc) lands with the first DMA
    wqkvT_d = nc.dram_tensor("wqkvT", [128, 6, 8, 128], BF16,
                             kind="ExternalInput").ap()
    wpT_d = nc.dram_tensor("wpT", [128, 8, C], BF16, kind="ExternalInput").ap()
    c_d = nc.dram_tensor("c128", [128, T], BF16, kind="ExternalInput").ap()
    s_d = nc.dram_tensor("s128", [128, T], BF16, kind="ExternalInput").ap()
    mask_d = nc.dram_tensor("mask", [128, 128], BF16, kind="ExternalInput").ap()
    lnw_d = nc.dram_tensor("ln_w", [C], F32, kind="ExternalInput").ap()
    lnb_d = nc.dram_tensor("ln_b", [C], F32, kind="ExternalInput").ap()
    boff_d = nc.dram_tensor("blk_off", [1, 1], mybir.dt.uint32,
                            kind="ExternalInput").ap()
    out_d = nc.dram_tensor("out", [T // 4, C], F32, kind="ExternalOutput").ap()

    with tile.TileContext(nc) as tc:
        with (
            tc.tile_pool(name="consts", bufs=1) as consts,
            tc.tile_pool(name="persist", bufs=1) as persist,
            tc.tile_pool(name="work", bufs=1) as work,
            tc.tile_pool(name="ps", bufs=1, space="PSUM") as ps,
            tc.tile_pool(name="dram", bufs=1, space="DRAM") as dram,
        ):
            # ---- hot-path inputs first: weights + first x tile ------------
            wqkv_sb = consts.tile([128, 8, 3 * DC], BF16)
            xt0 = work.tile([128, 8, TT], BF16, name="xt", bufs=2)
            nc.sync.dma_start(out=wqkv_sb[:, :, 0:128], in_=wqkvT_d[:, 0])
            nc.scalar.dma_start(out=xt0[:, 0, :], in_=xT_d[:, 0, 0, :])
            nc.sync.dma_start(out=wqkv_sb[:, :, 128:256], in_=wqkvT_d[:, 1])
            nc.scalar.dma_start(out=xt0[:, 1, :], in_=xT_d[:, 0, 1, :])
            nc.sync.dma_start(out=wqkv_sb[:, :, 256:384], in_=wqkvT_d[:, 2])
            nc.scalar.dma_start(out=xt0[:, 2:5, :], in_=xT_d[:, 0, 2:5, :])
            nc.sync.dma_start(out=wqkv_sb[:, :, 384:512], in_=wqkvT_d[:, 3])
            nc.scalar.dma_start(out=xt0[:, 5:8, :], in_=xT_d[:, 0, 5:8, :])
            nc.sync.dma_start(out=wqkv_sb[:, :, 512:640], in_=wqkvT_d[:, 4])
            nc.sync.dma_start(out=wqkv_sb[:, :, 640:768], in_=wqkvT_d[:, 5])
            # small tables on the scalar queue so they don't delay x/weights
            c_sb = consts.tile([128, T], BF16)
            nc.scalar.dma_start(out=c_sb[:], in_=c_d[:])
            s_sb = consts.tile([128, T], BF16)
            nc.scalar.dma_start(out=s_sb[:], in_=s_d[:])
            mask_sb = consts.tile([128, 128], BF16)
            nc.scalar.dma_start(out=mask_sb[:], in_=mask_d[:])
            magic_sb = consts.tile([128, 1], mybir.dt.uint32)
            nc.vector.memset(magic_sb[:], 0x5F3759DF)
            ones_sb = consts.tile([1, 64], BF16)
            nc.vector.memset(ones_sb[:], 1.0)
            boff_sb = consts.tile([1, 1], mybir.dt.uint32)
            nc.scalar.dma_start(out=boff_sb[:], in_=boff_d[:])
            if use_lnwb:
                lnw_sb = consts.tile([128, C], F32)
                nc.scalar.dma_start(out=lnw_sb[:], in_=bass.AP(
                    tensor=lnw_d.tensor, offset=lnw_d.offset,
                    ap=[[0, 128]] + list(lnw_d.ap)))
                lnb_sb = consts.tile([128, C], F32)
                nc.scalar.dma_start(out=lnb_sb[:], in_=bass.AP(
                    tensor=lnb_d.tensor, offset=lnb_d.offset,
                    ap=[[0, 128]] + list(lnb_d.ap)))
            warm_in = dram.tile([8, 32], F32, name="warm_in")
            warm_out = dram.tile([8, 32], F32, name="warm_out")
            warm_sb = consts.tile([8, 32], F32, name="warm_sb")
            nc.vector.memset(warm_sb[:], 0.0)
            nc.gpsimd.dma_start(out=warm_in[:], in_=warm_sb[:])
            nc.gpsimd.collective_compute(
                "AllToAll", mybir.AluOpType.bypass,
                replica_groups=[[0, 1, 2, 3, 4, 5, 6, 7]],
                ins=[warm_in.opt()], outs=[warm_out.opt()],
            )

            # ---- persistent activations -----------------------------------
            qrot = persist.tile([128, 2, T], BF16)   # [:, i, :] = heads 2i,2i+1
            krot = persist.tile([128, 2, T], BF16)
            v_sb = persist.tile([128, NTB, HPC, D + 1], BF16)
            nc.vector.memset(v_sb[:, :, :, D:D + 1], 1.0)

            # ---- phase 1 (interleaved): QKV projection + RoPE, and V ------
            # col blocks 0..1: q heads (01),(23); 2..3: k; cols 512:768: v
            def qkv_tile(tt):
                if tt == 0:
                    xt = xt0
                else:
                    xt = work.tile([128, 8, TT], BF16, name="xt", bufs=2)
                    nc.sync.dma_start(out=xt[:], in_=xT_d[:, tt, :, :])
                for cb in range(4):
                    qk_ps = ps.tile([128, TT], F32, name="qk_ps", tag="big1024",
                                    bufs=3, padded_shape=[128, 1024])
                    for kc in range(8):
                        nc.tensor.matmul(
                            qk_ps[:],
                            wqkv_sb[:, kc, cb * 128:(cb + 1) * 128],
                            xt[:, kc, :],
                            start=(kc == 0),
                            stop=(kc == 7),
                        )
                    qkT = work.tile([128, TT], BF16, name="qkT", bufs=2)
                    nc.vector.tensor_copy(qkT[:], qk_ps[:])
                    # rotation partner = half-swap within each 64-row head
                    # block, done by partition-slice DMAs (the sign flip is
                    # folded into the host-side s table). SBUF DMA sides
                    # need a single leading partition dim, hence 4 slices.
                    u_sb = work.tile([128, TT], BF16, name="u_sb", bufs=2)
                    for a in range(2):
                        for lohi in range(2):
                            dst0 = a * 64 + lohi * 32
                            src0 = a * 64 + (1 - lohi) * 32
                            nc.sync.dma_start(
                                out=u_sb[dst0:dst0 + 32, :],
                                in_=qkT[src0:src0 + 32, :])
                    dest = (qrot if cb < 2 else krot)[:, cb % 2,
                                                      tt * TT:(tt + 1) * TT]
                    csl = c_sb[:, tt * TT:(tt + 1) * TT]
                    ssl = s_sb[:, tt * TT:(tt + 1) * TT]
                    tmp = work.tile([128, TT], BF16, name="tmp", bufs=2)
                    tmp2 = work.tile([128, TT], BF16, name="tmp2", bufs=2)
                    nc.vector.tensor_mul(tmp[:], qkT[:], csl)
                    nc.vector.tensor_mul(tmp2[:], u_sb[:], ssl)
                    nc.vector.tensor_add(dest, tmp[:], tmp2[:])
                # V for the 4 token blocks of this token tile
                for tb in range(tt * 4, tt * 4 + 4):
                    v_ps = ps.tile([128, DC], F32, name="v_ps", tag="big1024",
                                   bufs=3, padded_shape=[128, 1024])
                    for kc in range(8):
                        nc.tensor.matmul(
                            v_ps[:],
                            xt[:, kc, (tb % 4) * 128:(tb % 4 + 1) * 128],
                            wqkv_sb[:, kc, 2 * DC:3 * DC],
                            start=(kc == 0),
                            stop=(kc == 7),
                        )
                    nc.scalar.copy(
                        v_sb[:, tb, :, 0:D],
                        v_ps[:].rearrange("p (h d) -> p h d", h=HPC),
                    )

            qkv_tile(0)

            # weights for c_proj (first needed ~halfway through attention)
            wp_sb = consts.tile([128, 8, C], BF16)
            nc.scalar.dma_start(out=wp_sb[:], in_=wpT_d[:])

            # ---- phase 2: per 512-token chunk: attention (4 heads) +
            #      chunked AllToAll; c_proj + LayerNorm in a tail loop -----
            scale = 1.0 / float(np.sqrt(D))
            boff_reg = nc.gpsimd.alloc_register("blkoff")
            nc.gpsimd.reg_load(boff_reg, boff_sb[0:1, 0:1])
            boff_val = nc.gpsimd.snap(boff_reg, donate=True, min_val=0,
                                      max_val=4)

            def load_yTg(a2a_pair):
                # separate tiles per source collective so cproj's partial-K
                # accumulation over the hp0 half never waits on the hp1
                # collective (range-exact deps)
                yTgs = []
                for hh in range(2):
                    yt = work.tile([128, 4, 128], BF16, name=f"yTg{hh}",
                                   bufs=2)
                    nc.gpsimd.dma_start(
                        out=yt[:],
                        in_=a2a_pair[hh][bass.ds(boff_val, 4)].rearrange(
                            "r p t -> p r t"),
                    )
                    yTgs.append(yt)
                return yTgs

            # even kc slots (hp0 data) first: they arrive one collective
            # earlier, so the partial-K accumulation can start sooner
            CP_KCS = [0, 2, 4, 6, 1, 3, 5, 7]

            def cproj_ln(yTgs, qt, tail=False):
                ln = work.tile([128, C], F32, name="ln", bufs=2)
                for oc in range(2):
                    co_ps = ps.tile([128, 512], F32, name="co_ps",
                                    tag="big1024", bufs=3,
                                    padded_shape=[128, 1024])
                    for j, kc in enumerate(CP_KCS):
                        nc.tensor.matmul(
                            co_ps[:],
                            yTgs[kc % 2][:, kc // 2, :],
                            wp_sb[:, kc, oc * 512:(oc + 1) * 512],
                            start=(j == 0),
                            stop=(j == 7),
                        )
                    nc.vector.tensor_copy(ln[:, oc * 512:(oc + 1) * 512],
                                          co_ps[:])
                stats = work.tile([128, 2, 6], F32, name="stats", bufs=2)
                nc.vector.bn_stats(stats[:, 0, :], ln[:, 0:512])
                nc.vector.bn_stats(stats[:, 1, :], ln[:, 512:1024])
                mv = work.tile([128, 2], F32, name="mv", bufs=2)
                nc.vector.bn_aggr(mv[:], stats[:])
                negmean = work.tile([128, 1], F32, name="negmean", bufs=2)
                nc.vector.tensor_scalar_mul(negmean[:], mv[:, 0:1], -1.0)
                # rstd = (var+eps)^-1/2 via bit-trick + 2 Newton steps, all
                # on DVE (the Sqrt ACT table would evict the Exp table).
                U32 = mybir.dt.uint32
                vpe = work.tile([128, 1], F32, name="vpe", bufs=2)
                nc.vector.tensor_scalar_add(vpe[:], mv[:, 1:2], EPS)
                r0 = work.tile([128, 1], F32, name="r0", bufs=2)
                nc.vector.tensor_single_scalar(
                    r0[:].bitcast(U32), vpe[:].bitcast(U32), 1,
                    op=mybir.AluOpType.logical_shift_right)
                nc.vector.tensor_tensor(
                    r0[:].bitcast(U32), magic_sb[:], r0[:].bitcast(U32),
                    op=mybir.AluOpType.subtract)
                rstd = r0
                for _ in range(2):
                    nt = work.tile([128, 1], F32, name="nt", bufs=4)
                    nc.vector.tensor_mul(nt[:], rstd[:], rstd[:])
                    nc.vector.tensor_mul(nt[:], nt[:], vpe[:])
                    nc.vector.tensor_scalar(
                        nt[:], nt[:], -0.5, 1.5,
                        op0=mybir.AluOpType.mult, op1=mybir.AluOpType.add)
                    nr = work.tile([128, 1], F32, name="nr", bufs=4)
                    nc.vector.tensor_mul(nr[:], rstd[:], nt[:])
                    rstd = nr
                nc.vector.tensor_scalar(
                    ln[:], ln[:], negmean[:], rstd[:],
                    op0=mybir.AluOpType.add, op1=mybir.AluOpType.mult,
                )
                if use_lnwb:
                    eng = nc.vector if tail else nc.gpsimd
                    eng.tensor_mul(ln[:], ln[:], lnw_sb[:])
                    eng.tensor_add(ln[:], ln[:], lnb_sb[:])
                nc.gpsimd.dma_start(out=out_d[qt * 128:(qt + 1) * 128, :],
                                    in_=ln[:])

            a2a_outs = []
            for qt in (0, 1, 2, 3):
                y_qt = work.tile([128, 2, TT], BF16, name="y_qt", bufs=2)
                a2a_ins = [dram.tile([8, 128, 128], BF16,
                                     name=f"a2a_in{qt}h{h}") for h in range(2)]
                a2a_out_pair = [dram.tile([8, 128, 128], BF16,
                                          name=f"a2a_out{qt}h{h}")
                                for h in range(2)]
                for hp in range(2):
                    hA, hB = 2 * hp, 2 * hp + 1
                    qsrc = qrot[:, hp, :]
                    ksrc = krot[:, hp, :]
                    y_psA = ps.tile([D + 1, TT], F32, name="y_psA",
                                    tag="ytag", bufs=2)
                    y_psB = ps.tile([D + 1, TT], F32, name="y_psB",
                                    tag="ytag", bufs=2)
                    n_kb = 4 * qt + 4
                    pend_av = None

                    def emit_av(kb, q_lo, ex):
                        nc.tensor.matmul(
                            y_psA[:, q_lo:],
                            v_sb[:, kb, hA, :],
                            ex[:, 0, q_lo:],
                            start=(kb == 0),
                            stop=(kb == n_kb - 1),
                            skip_group_check=True,
                        )
                        nc.tensor.matmul(
                            y_psB[:, q_lo:],
                            v_sb[:, kb, hB, :],
                            ex[:, 1, q_lo:],
                            start=(kb == 0),
                            stop=(kb == n_kb - 1),
                            skip_group_check=True,
                        )

                    for kb in range(n_kb):
                        s_off = kb - 4 * qt      # >= 0 -> diagonal block
                        q_lo = 128 * s_off if s_off > 0 else 0
                        sc_ps = ps.tile([128, 2, TT], F32, name="sc_ps",
                                        tag="big1024", bufs=3)
                        nc.tensor.matmul(
                            sc_ps[:, 0, q_lo:],
                            ksrc[0:64, kb * 128:(kb + 1) * 128],
                            qsrc[0:64, qt * TT + q_lo:(qt + 1) * TT],
                            start=True, stop=True,
                        )
                        nc.tensor.matmul(
                            sc_ps[:, 1, q_lo:],
                            ksrc[64:128, kb * 128:(kb + 1) * 128],
                            qsrc[64:128, qt * TT + q_lo:(qt + 1) * TT],
                            start=True, stop=True,
                        )
                        ex = work.tile([128, 2, TT], BF16, name="ex", bufs=3)
                        nc.scalar.activation(
                            ex[:, :, q_lo:], sc_ps[:, :, q_lo:],
                            mybir.ActivationFunctionType.Exp, scale=scale,
                        )
                        if s_off >= 0:
                            nc.vector.tensor_mul(
                                ex[:, 0, q_lo:q_lo + 128],
                                ex[:, 0, q_lo:q_lo + 128],
                                mask_sb[:],
                            )
                            nc.vector.tensor_mul(
                                ex[:, 1, q_lo:q_lo + 128],
                                ex[:, 1, q_lo:q_lo + 128],
                                mask_sb[:],
                            )
                        if pend_av is not None:
                            emit_av(*pend_av)
                        pend_av = (kb, q_lo, ex)
                    emit_av(*pend_av)
                    # normalize each head: y / denom (denom = row D of y_ps);
                    # the reciprocal is broadcast to 64 partitions by DMA
                    for half, y_ps in ((0, y_psA), (1, y_psB)):
                        den = work.tile([1, TT], BF16, name="den", bufs=2)
                        nc.vector.tensor_copy(den[:], y_ps[D:D + 1, :])
                        rep_ps = ps.tile([64, TT], F32, name="rep_ps",
                                         tag="big1024", bufs=3,
                                         padded_shape=[128, 1024])
                        nc.tensor.matmul(rep_ps[:], ones_sb[:], den[:],
                                         start=True, stop=True)
                        rrec = work.tile([64, TT], F32, name="rrec", bufs=2)
                        nc.vector.reciprocal_approx_fast(rrec[:], rep_ps[:])
                        nc.vector.tensor_mul(
                            y_qt[64 * half:64 * half + 64, hp, :],
                            y_ps[0:D, :],
                            rrec[:],
                        )
                    # ship this head-pair's y through its own AllToAll as
                    # soon as it is staged: the hp0 collective overlaps
                    # hp1's attention, halving the exposed tail
                    for half in range(2):
                        eng = nc.gpsimd if half == 0 else nc.sync
                        eng.dma_start(
                            out=a2a_ins[hp][4 * half:4 * half + 4].rearrange(
                                "r p t -> p r t"),
                            in_=y_qt[:, hp, :].rearrange(
                                "p (r t) -> p r t", r=4),
                        )
                    nc.gpsimd.collective_compute(
                        "AllToAll",
                        mybir.AluOpType.bypass,
                        replica_groups=[[0, 1, 2, 3, 4, 5, 6, 7]],
                        ins=[a2a_ins[hp].opt()],
                        outs=[a2a_out_pair[hp].opt()],
                    )
                a2a_outs.append(a2a_out_pair)
                if qt < 3:
                    qkv_tile(qt + 1)
            # tail: c_proj + LayerNorm per chunk, after ALL attention/qkv
            # work so the in-order PE queue never stalls on the collective
            # chain. All a2a triggers are already queued on gpsimd; the
            # yTg loads below queue after them.
            for qt in (0, 1, 2, 3):
                cproj_ln(load_yTg(a2a_outs[qt]), qt, tail=(qt == 3))

    nc.compile()
    return nc


_PERM64 = np.concatenate([np.arange(0, 64, 2), np.arange(1, 64, 2)])


def _host_prep(x, rope_freqs, W_attn, W_proj, ln_weight, ln_bias):
    """Build the 8 per-core input maps."""
    import ml_dtypes

    x = np.ascontiguousarray(np.asarray(x, dtype=np.float32))
    W_attn = np.asarray(W_attn, dtype=np.float32)
    W_proj = np.asarray(W_proj, dtype=np.float32)
    rope_freqs = np.asarray(rope_freqs, dtype=np.float32)
    ln_weight = np.ascontiguousarray(np.asarray(ln_weight, dtype=np.float32))
    ln_bias = np.ascontiguousarray(np.asarray(ln_bias, dtype=np.float32))

    # RoPE tables, [128, T]: row r uses pair frequency freqs[r % 32].
    # The sin table carries the rotation sign: rows 0-31 of each 64-row
    # head block multiply the (negated) swapped-in upper half.
    t = np.arange(T, dtype=np.float64)
    theta = t[None, :] * rope_freqs.astype(np.float64)[np.arange(128) % 32][:, None]
    c128 = np.cos(theta).astype(ml_dtypes.bfloat16)
    ssign = np.where((np.arange(128) % 64) < 32, -1.0, 1.0)[:, None]
    s128 = (np.sin(theta) * ssign).astype(ml_dtypes.bfloat16)

    mask = np.triu(np.ones((128, 128))).astype(ml_dtypes.bfloat16)

    # x as [128, tt, kc, 512]: per-SBUF-partition line contiguous per tile
    xT = [np.ascontiguousarray(
        x[b].T.reshape(8, 128, NTT, TT).transpose(1, 2, 0, 3)
    ).astype(ml_dtypes.bfloat16) for b in range(B)]
    # wp as [128, kc, 1024]
    wpT_full = np.ascontiguousarray(
        W_proj.T.reshape(8, 128, C).transpose(1, 0, 2)
    ).astype(ml_dtypes.bfloat16)

    in_maps = []
    for c in range(N_CORES):
        b, g = c // 4, c % 4
        heads = range(4 * g, 4 * g + 4)
        wq = np.concatenate([W_attn[h * D + _PERM64] for h in heads])
        wk = np.concatenate([W_attn[C + h * D + _PERM64] for h in heads])
        wv = W_attn[2 * C + 4 * g * D:2 * C + (4 * g + 4) * D]
        # [128, colblock, kc, 128]
        wqkvT = np.ascontiguousarray(
            np.concatenate([wq, wk, wv]).T.reshape(8, 128, 6, 128)
            .transpose(1, 2, 0, 3)
        ).astype(ml_dtypes.bfloat16)

        in_maps.append({
            "xT": xT[b],
            "wqkvT": wqkvT,
            "wpT": wpT_full,
            "c128": c128,
            "s128": s128,
            "mask": mask,
            "ln_w": ln_weight,
            "ln_b": ln_bias,
            "blk_off": np.array([[4 * b]], dtype=np.uint32),
        })
    return in_maps


_NC_CACHE = {}


def kernel(x, rope_freqs, W_attn, W_proj, ln_weight, ln_bias):
    use_lnwb = not (np.all(np.asarray(ln_weight) == 1.0)
                    and np.all(np.asarray(ln_bias) == 0.0))
    if use_lnwb not in _NC_CACHE:
        _NC_CACHE[use_lnwb] = build(use_lnwb)
    nc = _NC_CACHE[use_lnwb]
    in_maps = _host_prep(x, rope_freqs, W_attn, W_proj, ln_weight, ln_bias)
    res = None
    for attempt in range(3):
        try:
            res = run_bass_kernel_spmd(nc, in_maps,
                                       core_ids=list(range(N_CORES)))
            break
        except Exception:
            if attempt == 2:
                raise
    out = np.empty((B, T, C), dtype=np.float32)
    for c in range(N_CORES):
        b, g = c // 4, c % 4
        chunk = res.results[c]["out"]     # [512, 1024]: 4 chunks of 128 rows
        for qt in range(NTT):
            out[b, qt * 512 + g * 128:qt * 512 + g * 128 + 128, :] = \
                chunk[qt * 128:(qt + 1) * 128]
    return out


# revision 36
# speedup vs baseline: 1.4358x; 1.4358x over previous
"""Causal self-attention (QKV proj + RoPE + causal softmax attention +
output proj + LayerNorm) on 8 Trainium2 NeuronCores.

Sharding: core c handles batch b = c//4 and head group g = c%4 (4 of the
16 heads). Each core projects QKV for its heads from the full sequence,
runs flash-style attention in transposed ([dim, token]) layout, computes
its partial c_proj contribution, AllToAlls y head-pair slices in
128-token chunks (overlapped with later attention), LayerNorms each
received 128-token chunk, and returns [512, 1024] rows. Host assembles
the [2, 2048, 1024] output.

Precision: bf16 storage/matmuls throughout (f32 PSUM accumulate);
softmax/normalization statistics in f32.
"""
import sys

sys.path.insert(0, "/opt/trn_rl_repo")

import numpy as np

import concourse.bass as bass
import concourse.tile as tile
from concourse.tile_rust import add_dep_helper
from concourse import bacc, mybir
from concourse.bass_utils import run_bass_kernel_spmd

B, T, C, H = 2, 2048, 1024, 16
D = C // H              # 64
HPC = 4                 # heads per core
DC = HPC * D            # 256 local q/k/v dims per core
N_CORES = 8
EPS = 1e-5

F32 = mybir.dt.float32
BF16 = mybir.dt.bfloat16

TT = 512                # token tile (moving dim) for qkv/attention
NTT = T // TT           # 4
NTB = T // 128          # 16 token blocks


def build(use_lnwb):
    nc = bacc.Bacc("TRN2", target_bir_lowering=False, debug=False,
                   num_devices=N_CORES)

    # DMA-friendly layouts: per-SBUF-partition lines are contiguous in DRAM
    xT_d = nc.dram_tensor("xT", [128, NTT, 8, TT], BF16,
                          kind="ExternalInput").ap()
    # column-block major: [part, colblock(6), kc(8), col(128)] so the first
    # qkv column block (all kc) lands with the first DMA
    wqkvT_d = nc.dram_tensor("wqkvT", [128, 6, 8, 128], BF16,
                             kind="ExternalInput").ap()
    wpT_d = nc.dram_tensor("wpT", [128, 8, C], BF16, kind="ExternalInput").ap()
    c_d = nc.dram_tensor("c128", [128, T], BF16, kind="ExternalInput").ap()
    s_d = nc.dram_tensor("s128", [128, T], BF16, kind="ExternalInput").ap()
    mask_d = nc.dram_tensor("mask", [128, 128], BF16, kind="ExternalInput").ap()
    lnw_d = nc.dram_tensor("ln_w", [C], F32, kind="ExternalInput").ap()
    lnb_d = nc.dram_tensor("ln_b", [C], F32, kind="ExternalInput").ap()
    boff_d = nc.dram_tensor("blk_off", [1, 1], mybir.dt.uint32,
                            kind="ExternalInput").ap()
    out_d = nc.dram_tensor("out", [T // 4, C], F32, kind="ExternalOutput").ap()

    with tile.TileContext(nc) as tc:
        with (
            tc.tile_pool(name="consts", bufs=1) as consts,
            tc.tile_pool(name="persist", bufs=1) as persist,
            tc.tile_pool(name="work", bufs=1) as work,
            tc.tile_pool(name="ps", bufs=1, space="PSUM") as ps,
            tc.tile_pool(name="dram", bufs=1, space="DRAM") as dram,
        ):
            # ---- hot-path inputs first: weights + first x tile ------------
            wqkv_sb = consts.tile([128, 8, 3 * DC], BF16)
            xt0 = work.tile([128, 8, TT], BF16, name="xt", bufs=2)
            nc.sync.dma_start(out=wqkv_sb[:, :, 0:128], in_=wqkvT_d[:, 0])
            nc.scalar.dma_start(out=xt0[:, 0, :], in_=xT_d[:, 0, 0, :])
            nc.sync.dma_start(out=wqkv_sb[:, :, 128:256], in_=wqkvT_d[:, 1])
            nc.scalar.dma_start(out=xt0[:, 1, :], in_=xT_d[:, 0, 1, :])
            nc.sync.dma_start(out=wqkv_sb[:, :, 256:384], in_=wqkvT_d[:, 2])
            nc.scalar.dma_start(out=xt0[:, 2:5, :], in_=xT_d[:, 0, 2:5, :])
            nc.sync.dma_start(out=wqkv_sb[:, :, 384:512], in_=wqkvT_d[:, 3])
            nc.scalar.dma_start(out=xt0[:, 5:8, :], in_=xT_d[:, 0, 5:8, :])
            nc.sync.dma_start(out=wqkv_sb[:, :, 512:640], in_=wqkvT_d[:, 4])
            nc.sync.dma_start(out=wqkv_sb[:, :, 640:768], in_=wqkvT_d[:, 5])
            # small tables on the scalar queue so they don't delay x/weights
            c_sb = consts.tile([128, T], BF16)
            nc.scalar.dma_start(out=c_sb[:], in_=c_d[:])
            s_sb = consts.tile([128, T], BF16)
            nc.scalar.dma_start(out=s_sb[:], in_=s_d[:])
            mask_sb = consts.tile([128, 128], BF16)
            nc.scalar.dma_start(out=mask_sb[:], in_=mask_d[:])
            magic_sb = consts.tile([128, 1], mybir.dt.uint32)
            nc.vector.memset(magic_sb[:], 0x5F3759DF)
            ones_sb = consts.tile([1, 64], BF16)
            nc.vector.memset(ones_sb[:], 1.0)
            boff_sb = consts.tile([1, 1], mybir.dt.uint32)
            nc.scalar.dma_start(out=boff_sb[:], in_=boff_d[:])
            if use_lnwb:
                lnw_sb = consts.tile([128, C], F32)
                nc.scalar.dma_start(out=lnw_sb[:], in_=bass.AP(
                    tensor=lnw_d.tensor, offset=lnw_d.offset,
                    ap=[[0, 128]] + list(lnw_d.ap)))
                lnb_sb = consts.tile([128, C], F32)
                nc.scalar.dma_start(out=lnb_sb[:], in_=bass.AP(
                    tensor=lnb_d.tensor, offset=lnb_d.offset,
                    ap=[[0, 128]] + list(lnb_d.ap)))
            warm_in = dram.tile([8, 32], F32, name="warm_in")
            warm_out = dram.tile([8, 32], F32, name="warm_out")
            warm_sb = consts.tile([8, 32], F32, name="warm_sb")
            nc.vector.memset(warm_sb[:], 0.0)
            nc.gpsimd.dma_start(out=warm_in[:], in_=warm_sb[:])
            nc.gpsimd.collective_compute(
                "AllToAll", mybir.AluOpType.bypass,
                replica_groups=[[0, 1, 2, 3, 4, 5, 6, 7]],
                ins=[warm_in.opt()], outs=[warm_out.opt()],
            )

            # ---- persistent activations -----------------------------------
            qrot = persist.tile([128, 2, T], BF16)   # [:, i, :] = heads 2i,2i+1
            krot = persist.tile([128, 2, T], BF16)
            v_sb = persist.tile([128, NTB, HPC, D + 1], BF16)
            nc.vector.memset(v_sb[:, :, :, D:D + 1], 1.0)

            # ---- phase 1 (interleaved): QKV projection + RoPE, and V ------
            # col blocks 0..1: q heads (01),(23); 2..3: k; cols 512:768: v
            def qkv_tile(tt):
                if tt == 0:
                    xt = xt0
                else:
                    xt = work.tile([128, 8, TT], BF16, name="xt", bufs=2)
                    nc.sync.dma_start(out=xt[:], in_=xT_d[:, tt, :, :])
                for cb in range(4):
                    qk_ps = ps.tile([128, TT], F32, name="qk_ps", tag="big1024",
                                    bufs=3, padded_shape=[128, 1024])
                    for kc in range(8):
                        nc.tensor.matmul(
                            qk_ps[:],
                            wqkv_sb[:, kc, cb * 128:(cb + 1) * 128],
                            xt[:, kc, :],
                            start=(kc == 0),
                            stop=(kc == 7),
                        )
                    qkT = work.tile([128, TT], BF16, name="qkT", bufs=2)
                    nc.vector.tensor_copy(qkT[:], qk_ps[:])
                    # rotation partner = half-swap within each 64-row head
                    # block, done by partition-slice DMAs (the sign flip is
                    # folded into the host-side s table). SBUF DMA sides
                    # need a single leading partition dim, hence 4 slices.
                    u_sb = work.tile([128, TT], BF16, name="u_sb", bufs=2)
                    for a in range(2):
                        for lohi in range(2):
                            dst0 = a * 64 + lohi * 32
                            src0 = a * 64 + (1 - lohi) * 32
                            nc.sync.dma_start(
                                out=u_sb[dst0:dst0 + 32, :],
                                in_=qkT[src0:src0 + 32, :])
                    dest = (qrot if cb < 2 else krot)[:, cb % 2,
                                                      tt * TT:(tt + 1) * TT]
                    csl = c_sb[:, tt * TT:(tt + 1) * TT]
                    ssl = s_sb[:, tt * TT:(tt + 1) * TT]
                    tmp = work.tile([128, TT], BF16, name="tmp", bufs=2)
                    tmp2 = work.tile([128, TT], BF16, name="tmp2", bufs=2)
                    nc.vector.tensor_mul(tmp[:], qkT[:], csl)
                    nc.vector.tensor_mul(tmp2[:], u_sb[:], ssl)
                    nc.vector.tensor_add(dest, tmp[:], tmp2[:])
                # V for the 4 token blocks of this token tile
                for tb in range(tt * 4, tt * 4 + 4):
                    v_ps = ps.tile([128, DC], F32, name="v_ps", tag="big1024",
                                   bufs=3, padded_shape=[128, 1024])
                    for kc in range(8):
                        nc.tensor.matmul(
                            v_ps[:],
                            xt[:, kc, (tb % 4) * 128:(tb % 4 + 1) * 128],
                            wqkv_sb[:, kc, 2 * DC:3 * DC],
                            start=(kc == 0),
                            stop=(kc == 7),
                        )
                    nc.scalar.copy(
                        v_sb[:, tb, :, 0:D],
                        v_ps[:].rearrange("p (h d) -> p h d", h=HPC),
                    )

            qkv_tile(0)

            # weights for c_proj (first needed ~halfway through attention)
            wp_sb = consts.tile([128, 8, C], BF16)
            nc.scalar.dma_start(out=wp_sb[:], in_=wpT_d[:])

            # ---- phase 2: per 512-token chunk: attention (4 heads) +
            #      chunked AllToAll; c_proj + LayerNorm in a tail loop -----
            scale = 1.0 / float(np.sqrt(D))
            boff_reg = nc.gpsimd.alloc_register("blkoff")
            nc.gpsimd.reg_load(boff_reg, boff_sb[0:1, 0:1])
            boff_val = nc.gpsimd.snap(boff_reg, donate=True, min_val=0,
                                      max_val=4)

            def load_yTg(a2a_pair):
                if len(a2a_pair) == 1:
                    # single full-chunk collective: one combined tile
                    yTg = work.tile([128, 8, 128], BF16, name="yTg", bufs=2)
                    nc.gpsimd.dma_start(
                        out=yTg[:].rearrange("p (r hh) t -> p r hh t", hh=2),
                        in_=a2a_pair[0][bass.ds(boff_val, 4)].rearrange(
                            "r hh p t -> p r hh t"),
                    )
                    return [yTg]
                # split (last chunk): separate tiles per source collective so
                # cproj's hp0-half accumulation never waits on the hp1
                # collective (range-exact deps)
                yTgs = []
                for hh in range(2):
                    yt = work.tile([128, 4, 128], BF16, name=f"yTg{hh}",
                                   bufs=2)
                    nc.gpsimd.dma_start(
                        out=yt[:],
                        in_=a2a_pair[hh][bass.ds(boff_val, 4)].rearrange(
                            "r p t -> p r t"),
                    )
                    yTgs.append(yt)
                return yTgs

            # split form: even kc slots (hp0 data) first — they arrive one
            # collective earlier, so the partial-K accumulation starts sooner
            CP_KCS = [0, 2, 4, 6, 1, 3, 5, 7]

            def cproj_ln(yTgs, qt, tail=False):
                split = len(yTgs) == 2
                kcs = CP_KCS if split else list(range(8))
                ln = work.tile([128, C], F32, name="ln", bufs=2)
                for oc in range(2):
                    co_ps = ps.tile([128, 512], F32, name="co_ps",
                                    tag="big1024", bufs=3,
                                    padded_shape=[128, 1024])
                    for j, kc in enumerate(kcs):
                        src = (yTgs[kc % 2][:, kc // 2, :] if split
                               else yTgs[0][:, kc, :])
                        nc.tensor.matmul(
                            co_ps[:],
                            src,
                            wp_sb[:, kc, oc * 512:(oc + 1) * 512],
                            start=(j == 0),
                            stop=(j == 7),
                        )
                    nc.vector.tensor_copy(ln[:, oc * 512:(oc + 1) * 512],
                                          co_ps[:])
                stats = work.tile([128, 2, 6], F32, name="stats", bufs=2)
                nc.vector.bn_stats(stats[:, 0, :], ln[:, 0:512])
                nc.vector.bn_stats(stats[:, 1, :], ln[:, 512:1024])
                mv = work.tile([128, 2], F32, name="mv", bufs=2)
                nc.vector.bn_aggr(mv[:], stats[:])
                negmean = work.tile([128, 1], F32, name="negmean", bufs=2)
                nc.vector.tensor_scalar_mul(negmean[:], mv[:, 0:1], -1.0)
                # rstd = (var+eps)^-1/2 via bit-trick + 2 Newton steps, all
                # on DVE (the Sqrt ACT table would evict the Exp table).
                U32 = mybir.dt.uint32
                vpe = work.tile([128, 1], F32, name="vpe", bufs=2)
                nc.vector.tensor_scalar_add(vpe[:], mv[:, 1:2], EPS)
                r0 = work.tile([128, 1], F32, name="r0", bufs=2)
                nc.vector.tensor_single_scalar(
                    r0[:].bitcast(U32), vpe[:].bitcast(U32), 1,
                    op=mybir.AluOpType.logical_shift_right)
                nc.vector.tensor_tensor(
                    r0[:].bitcast(U32), magic_sb[:], r0[:].bitcast(U32),
                    op=mybir.AluOpType.subtract)
                rstd = r0
                for _ in range(2):
                    nt = work.tile([128, 1], F32, name="nt", bufs=4)
                    nc.vector.tensor_mul(nt[:], rstd[:], rstd[:])
                    nc.vector.tensor_mul(nt[:], nt[:], vpe[:])
                    nc.vector.tensor_scalar(
                        nt[:], nt[:], -0.5, 1.5,
                        op0=mybir.AluOpType.mult, op1=mybir.AluOpType.add)
                    nr = work.tile([128, 1], F32, name="nr", bufs=4)
                    nc.vector.tensor_mul(nr[:], rstd[:], nt[:])
                    rstd = nr
                nc.vector.tensor_scalar(
                    ln[:], ln[:], negmean[:], rstd[:],
                    op0=mybir.AluOpType.add, op1=mybir.AluOpType.mult,
                )
                if use_lnwb:
                    eng = nc.vector if tail else nc.gpsimd
                    eng.tensor_mul(ln[:], ln[:], lnw_sb[:])
                    eng.tensor_add(ln[:], ln[:], lnb_sb[:])
                nc.gpsimd.dma_start(out=out_d[qt * 128:(qt + 1) * 128, :],
                                    in_=ln[:])

            a2a_outs = []
            for qt in (0, 1, 2, 3):
                split = (qt == 3)
                y_qt = work.tile([128, 2, TT], BF16, name="y_qt", bufs=2)
                if split:
                    a2a_ins = [dram.tile([8, 128, 128], BF16,
                                         name=f"a2a_in{qt}h{h}")
                               for h in range(2)]
                    a2a_out_pair = [dram.tile([8, 128, 128], BF16,
                                              name=f"a2a_out{qt}h{h}")
                                    for h in range(2)]
                else:
                    a2a_in = dram.tile([8, 2, 128, 128], BF16,
                                       name=f"a2a_in{qt}")
                    a2a_out_pair = [dram.tile([8, 2, 128, 128], BF16,
                                              name=f"a2a_out{qt}")]
                for hp in range(2):
                    hA, hB = 2 * hp, 2 * hp + 1
                    qsrc = qrot[:, hp, :]
                    ksrc = krot[:, hp, :]
                    y_psA = ps.tile([D + 1, TT], F32, name="y_psA",
                                    tag="ytag", bufs=2)
                    y_psB = ps.tile([D + 1, TT], F32, name="y_psB",
                                    tag="ytag", bufs=2)
                    n_kb = 4 * qt + 4
                    pend_av = None

                    def emit_av(kb, q_lo, ex):
                        nc.tensor.matmul(
                            y_psA[:, q_lo:],
                            v_sb[:, kb, hA, :],
                            ex[:, 0, q_lo:],
                            start=(kb == 0),
                            stop=(kb == n_kb - 1),
                            skip_group_check=True,
                        )
                        nc.tensor.matmul(
                            y_psB[:, q_lo:],
                            v_sb[:, kb, hB, :],
                            ex[:, 1, q_lo:],
                            start=(kb == 0),
                            stop=(kb == n_kb - 1),
                            skip_group_check=True,
                        )

                    for kb in range(n_kb):
                        s_off = kb - 4 * qt      # >= 0 -> diagonal block
                        q_lo = 128 * s_off if s_off > 0 else 0
                        sc_ps = ps.tile([128, 2, TT], F32, name="sc_ps",
                                        tag="big1024", bufs=3)
                        nc.tensor.matmul(
                            sc_ps[:, 0, q_lo:],
                            ksrc[0:64, kb * 128:(kb + 1) * 128],
                            qsrc[0:64, qt * TT + q_lo:(qt + 1) * TT],
                            start=True, stop=True,
                        )
                        nc.tensor.matmul(
                            sc_ps[:, 1, q_lo:],
                            ksrc[64:128, kb * 128:(kb + 1) * 128],
                            qsrc[64:128, qt * TT + q_lo:(qt + 1) * TT],
                            start=True, stop=True,
                        )
                        ex = work.tile([128, 2, TT], BF16, name="ex", bufs=3)
                        nc.scalar.activation(
                            ex[:, :, q_lo:], sc_ps[:, :, q_lo:],
                            mybir.ActivationFunctionType.Exp, scale=scale,
                        )
                        if s_off >= 0:
                            nc.vector.tensor_mul(
                                ex[:, 0, q_lo:q_lo + 128],
                                ex[:, 0, q_lo:q_lo + 128],
                                mask_sb[:],
                            )
                            nc.vector.tensor_mul(
                                ex[:, 1, q_lo:q_lo + 128],
                                ex[:, 1, q_lo:q_lo + 128],
                                mask_sb[:],
                            )
                        if pend_av is not None:
                            emit_av(*pend_av)
                        pend_av = (kb, q_lo, ex)
                    emit_av(*pend_av)
                    # normalize each head: y / denom (denom = row D of y_ps);
                    # the reciprocal is broadcast to 64 partitions by DMA
                    for half, y_ps in ((0, y_psA), (1, y_psB)):
                        den = work.tile([1, TT], BF16, name="den", bufs=2)
                        nc.vector.tensor_copy(den[:], y_ps[D:D + 1, :])
                        rep_ps = ps.tile([64, TT], F32, name="rep_ps",
                                         tag="big1024", bufs=3,
                                         padded_shape=[128, 1024])
                        nc.tensor.matmul(rep_ps[:], ones_sb[:], den[:],
                                         start=True, stop=True)
                        rrec = work.tile([64, TT], F32, name="rrec", bufs=2)
                        nc.vector.reciprocal_approx_fast(rrec[:], rep_ps[:])
                        nc.vector.tensor_mul(
                            y_qt[64 * half:64 * half + 64, hp, :],
                            y_ps[0:D, :],
                            rrec[:],
                        )
                    # stage this head-pair's y for the AllToAll: every core
                    # gets these 128 y-dims for its own 128-token slice.
                    # The last chunk ships per-hp so the hp0 collective
                    # overlaps hp1's attention, halving the exposed tail.
                    for half in range(2):
                        eng = nc.gpsimd if half == 0 else nc.sync
                        dst = (a2a_ins[hp][4 * half:4 * half + 4] if split
                               else a2a_in[4 * half:4 * half + 4, hp])
                        eng.dma_start(
                            out=dst.rearrange("r p t -> p r t"),
                            in_=y_qt[:, hp, :].rearrange(
                                "p (r t) -> p r t", r=4),
                        )
                    if split:
                        nc.gpsimd.collective_compute(
                            "AllToAll",
                            mybir.AluOpType.bypass,
                            replica_groups=[[0, 1, 2, 3, 4, 5, 6, 7]],
                            ins=[a2a_ins[hp].opt()],
                            outs=[a2a_out_pair[hp].opt()],
                        )
                if not split:
                    nc.gpsimd.collective_compute(
                        "AllToAll",
                        mybir.AluOpType.bypass,
                        replica_groups=[[0, 1, 2, 3, 4, 5, 6, 7]],
                        ins=[a2a_in.opt()],
                        outs=[a2a_out_pair[0].opt()],
                    )
                a2a_outs.append(a2a_out_pair)
                if qt < 3:
                    qkv_tile(qt + 1)
            # tail: c_proj + LayerNorm per chunk, after ALL attention/qkv
            # work so the in-order PE queue never stalls on the collective
            # chain. All a2a triggers are already queued on gpsimd; the
            # yTg loads below queue after them.
            for qt in (0, 1, 2, 3):
                cproj_ln(load_yTg(a2a_outs[qt]), qt, tail=(qt == 3))

    nc.compile()
    return nc


_PERM64 = np.concatenate([np.arange(0, 64, 2), np.arange(1, 64, 2)])


def _host_prep(x, rope_freqs, W_attn, W_proj, ln_weight, ln_bias):
    """Build the 8 per-core input maps."""
    import ml_dtypes

    x = np.ascontiguousarray(np.asarray(x, dtype=np.float32))
    W_attn = np.asarray(W_attn, dtype=np.float32)
    W_proj = np.asarray(W_proj, dtype=np.float32)
    rope_freqs = np.asarray(rope_freqs, dtype=np.float32)
    ln_weight = np.ascontiguousarray(np.asarray(ln_weight, dtype=np.float32))
    ln_bias = np.ascontiguousarray(np.asarray(ln_bias, dtype=np.float32))

    # RoPE tables, [128, T]: row r uses pair frequency freqs[r % 32].
    # The sin table carries the rotation sign: rows 0-31 of each 64-row
    # head block multiply the (negated) swapped-in upper half.
    t = np.arange(T, dtype=np.float64)
    theta = t[None, :] * rope_freqs.astype(np.float64)[np.arange(128) % 32][:, None]
    c128 = np.cos(theta).astype(ml_dtypes.bfloat16)
    ssign = np.where((np.arange(128) % 64) < 32, -1.0, 1.0)[:, None]
    s128 = (np.sin(theta) * ssign).astype(ml_dtypes.bfloat16)

    mask = np.triu(np.ones((128, 128))).astype(ml_dtypes.bfloat16)

    # x as [128, tt, kc, 512]: per-SBUF-partition line contiguous per tile
    xT = [np.ascontiguousarray(
        x[b].T.reshape(8, 128, NTT, TT).transpose(1, 2, 0, 3)
    ).astype(ml_dtypes.bfloat16) for b in range(B)]
    # wp as [128, kc, 1024]
    wpT_full = np.ascontiguousarray(
        W_proj.T.reshape(8, 128, C).transpose(1, 0, 2)
    ).astype(ml_dtypes.bfloat16)

    in_maps = []
    for c in range(N_CORES):
        b, g = c // 4, c % 4
        heads = range(4 * g, 4 * g + 4)
        wq = np.concatenate([W_attn[h * D + _PERM64] for h in heads])
        wk = np.concatenate([W_attn[C + h * D + _PERM64] for h in heads])
        wv = W_attn[2 * C + 4 * g * D:2 * C + (4 * g + 4) * D]
        # [128, colblock, kc, 128]
        wqkvT = np.ascontiguousarray(
            np.concatenate([wq, wk, wv]).T.reshape(8, 128, 6, 128)
            .transpose(1, 2, 0, 3)
        ).astype(ml_dtypes.bfloat16)

        in_maps.append({
            "xT": xT[b],
            "wqkvT": wqkvT,
            "wpT": wpT_full,
            "c128": c128,
            "s128": s128,
            "mask": mask,
            "ln_w": ln_weight,
            "ln_b": ln_bias,
            "blk_off": np.array([[4 * b]], dtype=np.uint32),
        })
    return in_maps


_NC_CACHE = {}


def kernel(x, rope_freqs, W_attn, W_proj, ln_weight, ln_bias):
    use_lnwb = not (np.all(np.asarray(ln_weight) == 1.0)
                    and np.all(np.asarray(ln_bias) == 0.0))
    if use_lnwb not in _NC_CACHE:
        _NC_CACHE[use_lnwb] = build(use_lnwb)
    nc = _NC_CACHE[use_lnwb]
    in_maps = _host_prep(x, rope_freqs, W_attn, W_proj, ln_weight, ln_bias)
    res = None
    for attempt in range(3):
        try:
            res = run_bass_kernel_spmd(nc, in_maps,
                                       core_ids=list(range(N_CORES)))
            break
        except Exception:
            if attempt == 2:
                raise
    out = np.empty((B, T, C), dtype=np.float32)
    for c in range(N_CORES):
        b, g = c // 4, c % 4
        chunk = res.results[c]["out"]     # [512, 1024]: 4 chunks of 128 rows
        for qt in range(NTT):
            out[b, qt * 512 + g * 128:qt * 512 + g * 128 + 128, :] = \
                chunk[qt * 128:(qt + 1) * 128]
    return out
